# revision 17
# baseline (speedup 1.0000x reference)
"""AFPM (adaptive per-patch modulation) kernel for 8 TRN2 NeuronCores.

Reference computation (B=8, C=64, H=W=512, K=8, HID=64):
  - d[l]: normalized distance of each 8x8 patch center from image center
  - pk[l, kk] / pb[l]: tiny MLPs of d (host-precomputable, data-independent)
  - feats[b,c,l] = sum_kk patches[b,c,kk,l] * pk[l,kk] + pb[l]
  - feats2 = conv_w @ feats + conv_b           (1x1 conv over channels)
  - out patches = patches * feats2[:, :, None, :]

Sharding (v4): core i handles patch-rows i*8..i*8+7 for ALL 8 images.
Macro-tile (t, h) = patch-row t (of 8), image-pair-pair h (of 2):
partitions = (u, c); free = (tl, dy, pw, dx) = 8192 covering image
pairs v = 2h, 2h+1.  Halved instruction count vs one-pair tiles.

Engine split per macro-tile (x traffic in bf16; 4 MiB DMA per macro):
  DMA  in   : xb halves, rings alternate sync/scalar
  DVE  mul  : PROD = xb * PKREP[t] (bcast over tl)   TT 2x, ~4.6us
  Pool L1   : dy 8->4 halving add, bf16             (offloads DVE)
  DVE  L2   : dy 4->2, writes (tl,pw,dy2,dx) layout  TT 2x
  DVE  red  : f[p,(tl pw)] = reduce_{dy2,dx}  f32    1x
  PE   g    : g = bd.T @ f (+) w2.T @ [pb;1]         (PSUM accum)
  ACT  gexp : bf16(g) expanded over dx
  DVE  out  : OUT = xb * bcast(gexp) into PROD buf   TT 2x
  DMA  out  : halves on the two rings

pkr rows ship DMA-replicated ([NR,128,FD] bf16, 1 MiB per row, each
reused by both macro-tiles of the row = 4 pair-tiles).
"""

import math
import sys

import numpy as np

for _p in ("/opt/trn_rl_repo",):
    if _p not in sys.path:
        sys.path.insert(0, _p)

import concourse.bass as bass
import concourse.tile as tile
from concourse import bacc, mybir
from concourse.bass_utils import run_bass_kernel_spmd

B, C, H, W, K, HID = 8, 64, 512, 512, 8, 64
NH, NW = H // K, W // K          # 64, 64
L = NH * NW                      # 4096
NR = 8                           # patch-rows per core
NV = 4                           # image-pairs per row
NT = NR * NV                     # 32 pair-tiles per core
FD = K * W                       # 4096 free dim per pair-tile
MFD = 2 * FD                     # 8192 per macro-tile
F32 = mybir.dt.float32
BF16 = mybir.dt.bfloat16

_ERF = np.frompyfunc(math.erf, 1, 1)


def _gelu(x):
    x = np.asarray(x, np.float64)
    return 0.5 * x * (1.0 + _ERF(x / math.sqrt(2.0)).astype(np.float64))


def _host_tables(w1k, b1k, w2k, b2k, w1b, b1b, w2b, b2b, conv_w, conv_b):
    """pk/pb via the tiny MLPs; packed as PKR [NH, FD] plus fold consts."""
    cy = cx = H / 2.0
    max_d = math.sqrt(cy * cy + cx * cx)
    py = np.arange(NH, dtype=np.float64) * K + K / 2.0
    px = np.arange(NW, dtype=np.float64) * K + K / 2.0
    d = np.sqrt((py - cy)[:, None] ** 2 + (px - cx)[None, :] ** 2) / max_d
    d = d.reshape(L, 1)

    pk = _gelu(d @ w1k.astype(np.float64) + b1k) @ w2k.astype(np.float64) + b2k
    pb = (_gelu(d @ w1b.astype(np.float64) + b1b) @ w2b.astype(np.float64) + b2b)[:, 0]

    import ml_dtypes

    # PKR[ph, dy*W + pw*K + dx] = pk[ph*NW + pw, dy*K + dx]  (bf16 on device)
    pkr = (
        pk.reshape(NH, NW, K, K).transpose(0, 2, 1, 3).reshape(NH, FD)
    ).astype(ml_dtypes.bfloat16)

    # g = bd.T @ f  +  w2.T @ [pb_row; 1]   (rank-2 fold of pb and conv_b)
    cw1 = conv_w.astype(np.float64).sum(axis=1)
    w2 = np.stack([np.tile(cw1, 2), np.tile(conv_b.astype(np.float64), 2)]).astype(
        np.float32
    )  # [2, 128]

    bd = np.zeros((128, 128), np.float32)
    bd[0:C, 0:C] = conv_w.T
    bd[C:128, C:128] = conv_w.T
    return pkr, pb, w2, bd, pk


def build_program():
    nc = bacc.Bacc("TRN2", target_bir_lowering=False, debug=False, num_devices=8)
    x_d = nc.dram_tensor("x", [NV, 128, NR * K, W], BF16, kind="ExternalInput")
    pkrz_d = nc.dram_tensor("pkrz", [NR, 128, FD], BF16, kind="ExternalInput")
    pbx_d = nc.dram_tensor("pbx", [2, NR * 128], F32, kind="ExternalInput")
    w2_d = nc.dram_tensor("w2", [2, 128], F32, kind="ExternalInput")
    bd_d = nc.dram_tensor("bd", [128, 128], F32, kind="ExternalInput")
    out_d = nc.dram_tensor("out", [NV, 128, NR * K, W], BF16, kind="ExternalOutput")

    # [t, v, p=(u c), dy, w] views of the DRAM image slices
    xr = x_d.ap().rearrange("v p (t dy) w -> t v p dy w", dy=K)
    outr = out_d.ap().rearrange("v p (t dy) w -> t v p dy w", dy=K)

    with tile.TileContext(nc) as tc:
        with (
            tc.tile_pool(name="const", bufs=1) as constp,
            tc.tile_pool(name="xbp", bufs=3) as xbp,
            tc.tile_pool(name="prodp", bufs=3) as prodp,
            tc.tile_pool(name="t1p", bufs=2) as t1p,
            tc.tile_pool(name="t2p", bufs=2) as t2p,
            tc.tile_pool(name="t3p", bufs=2) as t3p,
            tc.tile_pool(name="pkrepp", bufs=2) as pkrepp,
            tc.tile_pool(name="smallp", bufs=4) as smallp,
            tc.tile_pool(name="gpsum", bufs=3, space="PSUM") as gpsum,
        ):
            pbx = constp.tile([2, NR * 128], F32)
            nc.sync.dma_start(pbx[:], pbx_d[:])
            w2t = constp.tile([2, 128], F32)
            nc.sync.dma_start(w2t[:], w2_d[:])
            bdt = constp.tile([128, 128], F32)
            nc.sync.dma_start(bdt[:], bd_d[:])

            for t in range(NR):
                pkrep = pkrepp.tile([128, FD], BF16)
                (nc.sync if t % 2 == 0 else nc.scalar).dma_start(
                    pkrep[:], pkrz_d[t]
                )
                for h in range(2):
                    ring_a = nc.sync if h == 0 else nc.scalar
                    ring_b = nc.scalar if h == 0 else nc.sync

                    xb = xbp.tile([128, MFD], BF16)
                    ring_a.dma_start(
                        xb[:, 0:FD].rearrange("p (dy w) -> p dy w", dy=K),
                        xr[t, 2 * h],
                    )
                    ring_b.dma_start(
                        xb[:, FD:MFD].rearrange("p (dy w) -> p dy w", dy=K),
                        xr[t, 2 * h + 1],
                    )

                    # PROD = xb * pkrep  (pkrep broadcast over the tl pair)
                    prod = prodp.tile([128, MFD], BF16)
                    pr3 = prod.rearrange("p (tl f) -> p tl f", tl=2)
                    xb3 = xb.rearrange("p (tl f) -> p tl f", tl=2)
                    pk3 = pkrep.rearrange("p (a f) -> p a f", a=1)
                    xb3b, pk3b = bass.broadcast_tensor_aps(xb3, pk3)
                    nc.vector.tensor_tensor(pr3, xb3b, pk3b, op=mybir.AluOpType.mult)

                    with nc.allow_low_precision("pairwise bf16 tree adds"):
                        # L1 on Pool: dy 8 -> 4
                        t1 = t1p.tile([128, MFD // 2], BF16)
                        pr4 = prod.rearrange(
                            "p (tl dy q) -> p tl dy q", tl=2, dy=K
                        )
                        t14 = t1.rearrange(
                            "p (tl dy q) -> p tl dy q", tl=2, dy=K // 2
                        )
                        nc.gpsimd.tensor_tensor(
                            t14,
                            pr4[:, :, 0 : K // 2, :],
                            pr4[:, :, K // 2 : K, :],
                            op=mybir.AluOpType.add,
                        )
                        # L2a on DVE: dy 4 -> 2 (natural layout)
                        t2 = t2p.tile([128, MFD // 4], BF16)
                        t14b = t1.rearrange(
                            "p (tl dy q) -> p tl dy q", tl=2, dy=4
                        )
                        t24 = t2.rearrange(
                            "p (tl dy2 q) -> p tl dy2 q", tl=2, dy2=2
                        )
                        nc.vector.tensor_tensor(
                            t24,
                            t14b[:, :, 0:2, :],
                            t14b[:, :, 2:4, :],
                            op=mybir.AluOpType.add,
                        )
                        # L2b on DVE: dy 2 -> 1
                        t3 = t3p.tile([128, MFD // 8], BF16)
                        t34 = t3.rearrange("p (tl a q) -> p tl a q", tl=2, a=1)
                        nc.vector.tensor_tensor(
                            t34,
                            t24[:, :, 0:1, :],
                            t24[:, :, 1:2, :],
                            op=mybir.AluOpType.add,
                        )

                    # f[p, (tl pw)] = sum_dx t3  (single X-axis reduce)
                    f = smallp.tile([128, 128], F32)
                    nc.vector.tensor_reduce(
                        f[:],
                        t3.rearrange("p (q dx) -> p q dx", dx=K),
                        axis=mybir.AxisListType.X,
                        op=mybir.AluOpType.add,
                    )

                    g = gpsum.tile([128, 128], F32)
                    nc.tensor.matmul(g[:], bdt[:], f[:], start=True, stop=False)
                    nc.tensor.matmul(
                        g[:],
                        w2t[:],
                        pbx[:, t * 128 : (t + 1) * 128],
                        start=False,
                        stop=True,
                    )

                    # cast g to bf16 expanded over dx (dense 512-elem inner run)
                    gexp = smallp.tile([128, 128 * K], BF16, tag="gexp")
                    ge3 = gexp.rearrange("p (q dx) -> p q dx", dx=K)
                    gs3 = g.rearrange("p (q a) -> p q a", a=1)
                    ge3b, gs3b = bass.broadcast_tensor_aps(ge3, gs3)
                    nc.scalar.copy(ge3b, gs3b)

                    # OUT = xb * bcast(gexp) over dy, bf16, into PROD's buffer
                    o4 = prod.rearrange("p (tl dy q) -> p tl dy q", tl=2, dy=K)
                    x4 = xb.rearrange("p (tl dy q) -> p tl dy q", tl=2, dy=K)
                    g4 = gexp.rearrange("p (tl a q) -> p tl a q", tl=2, a=1)
                    x4b, g4b = bass.broadcast_tensor_aps(x4, g4)
                    nc.vector.tensor_tensor(o4, x4b, g4b, op=mybir.AluOpType.mult)

                    ring_a.dma_start(
                        outr[t, 2 * h],
                        prod[:, 0:FD].rearrange("p (dy w) -> p dy w", dy=K),
                    )
                    ring_b.dma_start(
                        outr[t, 2 * h + 1],
                        prod[:, FD:MFD].rearrange("p (dy w) -> p dy w", dy=K),
                    )

    nc.compile()
    return nc


_PROGRAM = None
LAST_RESULT = None


def make_in_maps(x, pkr, pb, w2, bd):
    import ml_dtypes

    in_maps = []
    for i in range(8):
        r0 = i * NR
        x_core = (
            np.ascontiguousarray(x[:, :, r0 * K : (r0 + NR) * K, :])
            .astype(ml_dtypes.bfloat16)
            .reshape(NV, 128, NR * K, W)
        )
        pkrz = np.ascontiguousarray(
            np.broadcast_to(pkr[r0 : r0 + NR, None, :], (NR, 128, FD))
        )
        pbx = np.empty((2, NR * 128), np.float32)
        pbx[0] = np.tile(
            pb[r0 * NW : (r0 + NR) * NW].reshape(NR, 1, NW), (1, 2, 1)
        ).reshape(NR * 128)
        pbx[1] = 1.0
        in_maps.append(
            {"x": x_core, "pkrz": pkrz, "pbx": pbx, "w2": w2, "bd": bd}
        )
    return in_maps


def kernel(**inputs):
    global _PROGRAM, LAST_RESULT
    x = np.ascontiguousarray(np.asarray(inputs["x"], dtype=np.float32))
    pkr, pb, w2, bd, pk = _host_tables(
        *[
            np.asarray(inputs[k], dtype=np.float32)
            for k in (
                "w1k", "b1k", "w2k", "b2k",
                "w1b", "b1b", "w2b", "b2b",
                "conv_w", "conv_b",
            )
        ]
    )
    if _PROGRAM is None:
        _PROGRAM = build_program()
    nc = _PROGRAM

    in_maps = make_in_maps(x, pkr, pb, w2, bd)

    conv_w = np.asarray(inputs["conv_w"], np.float64)
    conv_b = np.asarray(inputs["conv_b"], np.float64)

    def _spot_check(out):
        """Verify a sample of patches against the exact host formula;
        catches the rare silent device corruption (bf16 path ~0.4%/elem)."""
        rng = np.random.default_rng(1234)
        worst = 0.0
        for _ in range(32):
            b = int(rng.integers(B))
            ph = int(rng.integers(NH))
            pw = int(rng.integers(NW))
            l = ph * NW + pw
            patch = x[b, :, ph * K : (ph + 1) * K, pw * K : (pw + 1) * K]
            patch = patch.reshape(C, K * K).astype(np.float64)
            feats = patch @ pk[l] + pb[l]
            g = conv_w @ feats + conv_b
            exp = patch * g[:, None]
            got = out[b, :, ph * K : (ph + 1) * K, pw * K : (pw + 1) * K]
            got = got.reshape(C, K * K).astype(np.float64)
            denom = np.linalg.norm(exp) + 1e-30
            worst = max(worst, float(np.linalg.norm(got - exp) / denom))
        return worst

    res = None
    for attempt in range(4):
        try:
            res = run_bass_kernel_spmd(nc, in_maps, list(range(8)))
        except Exception:
            if attempt == 3:
                raise
            continue
        out = np.empty((B, C, H, W), np.float32)
        for i in range(8):
            r0 = i * NR
            out[:, :, r0 * K : (r0 + NR) * K, :] = (
                res.results[i]["out"].astype(np.float32).reshape(B, C, NR * K, W)
            )
        err = _spot_check(out)
        if err < 0.05:
            break
        if attempt == 3:
            raise RuntimeError(f"device output failed spot check ({err:.3f})")
    LAST_RESULT = res
    return out


# revision 19
# speedup vs baseline: 1.4140x; 1.4140x over previous
"""AFPM (adaptive per-patch modulation) kernel for 8 TRN2 NeuronCores.

Reference computation (B=8, C=64, H=W=512, K=8, HID=64):
  - d[l]: normalized distance of each 8x8 patch center from image center
  - pk[l, kk] / pb[l]: tiny MLPs of d (host-precomputable, data-independent)
  - feats[b,c,l] = sum_kk patches[b,c,kk,l] * pk[l,kk] + pb[l]
  - feats2 = conv_w @ feats + conv_b           (1x1 conv over channels)
  - out patches = patches * feats2[:, :, None, :]

Sharding (v4): core i handles patch-rows i*8..i*8+7 for ALL 8 images.
Macro-tile (t, h) = patch-row t (of 8), image-pair-pair h (of 2):
partitions = (u, c); free = (tl, dy, pw, dx) = 8192 covering image
pairs v = 2h, 2h+1.  Halved instruction count vs one-pair tiles.

Engine split per macro-tile (x traffic in bf16; 4 MiB DMA per macro):
  DMA  in   : xb halves, rings alternate sync/scalar
  DVE  mul  : PROD = xb * PKREP[t] (bcast over tl)   TT 2x, ~4.6us
  Pool L1   : dy 8->4 halving add, bf16             (offloads DVE)
  DVE  L2   : dy 4->2, writes (tl,pw,dy2,dx) layout  TT 2x
  DVE  red  : f[p,(tl pw)] = reduce_{dy2,dx}  f32    1x
  PE   g    : g = bd.T @ f (+) w2.T @ [pb;1]         (PSUM accum)
  ACT  gexp : bf16(g) expanded over dx
  DVE  out  : OUT = xb * bcast(gexp) into PROD buf   TT 2x
  DMA  out  : halves on the two rings

pkr rows ship DMA-replicated ([NR,128,FD] bf16, 1 MiB per row, each
reused by both macro-tiles of the row = 4 pair-tiles).
"""

import math
import sys

import numpy as np

for _p in ("/opt/trn_rl_repo",):
    if _p not in sys.path:
        sys.path.insert(0, _p)

import concourse.bass as bass
import concourse.tile as tile
from concourse import bacc, mybir
from concourse.bass_utils import run_bass_kernel_spmd

B, C, H, W, K, HID = 8, 64, 512, 512, 8, 64
NH, NW = H // K, W // K          # 64, 64
L = NH * NW                      # 4096
NR = 8                           # patch-rows per core
NV = 4                           # image-pairs per row
NT = NR * NV                     # 32 pair-tiles per core
FD = K * W                       # 4096 free dim per pair-tile
MFD = 2 * FD                     # 8192 per macro-tile
F32 = mybir.dt.float32
BF16 = mybir.dt.bfloat16

_ERF = np.frompyfunc(math.erf, 1, 1)


def _gelu(x):
    x = np.asarray(x, np.float64)
    return 0.5 * x * (1.0 + _ERF(x / math.sqrt(2.0)).astype(np.float64))


def _host_tables(w1k, b1k, w2k, b2k, w1b, b1b, w2b, b2b, conv_w, conv_b):
    """pk/pb via the tiny MLPs; packed as PKR [NH, FD] plus fold consts."""
    cy = cx = H / 2.0
    max_d = math.sqrt(cy * cy + cx * cx)
    py = np.arange(NH, dtype=np.float64) * K + K / 2.0
    px = np.arange(NW, dtype=np.float64) * K + K / 2.0
    d = np.sqrt((py - cy)[:, None] ** 2 + (px - cx)[None, :] ** 2) / max_d
    d = d.reshape(L, 1)

    pk = _gelu(d @ w1k.astype(np.float64) + b1k) @ w2k.astype(np.float64) + b2k
    pb = (_gelu(d @ w1b.astype(np.float64) + b1b) @ w2b.astype(np.float64) + b2b)[:, 0]

    import ml_dtypes

    # PKR[ph, dy*W + pw*K + dx] = pk[ph*NW + pw, dy*K + dx]  (bf16 on device)
    pkr = (
        pk.reshape(NH, NW, K, K).transpose(0, 2, 1, 3).reshape(NH, FD)
    ).astype(ml_dtypes.bfloat16)

    # g = bd.T @ f  +  w2.T @ [pb_row; 1]   (rank-2 fold of pb and conv_b)
    cw1 = conv_w.astype(np.float64).sum(axis=1)
    w2 = np.stack([np.tile(cw1, 2), np.tile(conv_b.astype(np.float64), 2)]).astype(
        np.float32
    )  # [2, 128]

    bd = np.zeros((128, 128), np.float32)
    bd[0:C, 0:C] = conv_w.T
    bd[C:128, C:128] = conv_w.T
    return pkr, pb, w2, bd, pk


def build_program():
    nc = bacc.Bacc("TRN2", target_bir_lowering=False, debug=False, num_devices=8)
    x_d = nc.dram_tensor("x", [NV, 128, NR * K, W], BF16, kind="ExternalInput")
    pkrz_d = nc.dram_tensor("pkrz", [NR, 128, FD], BF16, kind="ExternalInput")
    pbx_d = nc.dram_tensor("pbx", [2, NR * 128], F32, kind="ExternalInput")
    w2_d = nc.dram_tensor("w2", [2, 128], F32, kind="ExternalInput")
    bd_d = nc.dram_tensor("bd", [128, 128], F32, kind="ExternalInput")
    out_d = nc.dram_tensor("out", [NV, 128, NR * K, W], BF16, kind="ExternalOutput")

    # [t, v, p=(u c), dy, w] views of the DRAM image slices
    xr = x_d.ap().rearrange("v p (t dy) w -> t v p dy w", dy=K)
    outr = out_d.ap().rearrange("v p (t dy) w -> t v p dy w", dy=K)

    with tile.TileContext(nc) as tc:
        with (
            tc.tile_pool(name="const", bufs=1) as constp,
            tc.tile_pool(name="xbp", bufs=4) as xbp,
            tc.tile_pool(name="prodp", bufs=4) as prodp,
            tc.tile_pool(name="t1p", bufs=2) as t1p,
            tc.tile_pool(name="t2p", bufs=2) as t2p,
            tc.tile_pool(name="t3p", bufs=2) as t3p,
            tc.tile_pool(name="pkrepp", bufs=2) as pkrepp,
            tc.tile_pool(name="smallp", bufs=4) as smallp,
            tc.tile_pool(name="gpsum", bufs=3, space="PSUM") as gpsum,
        ):
            pbx = constp.tile([2, NR * 128], F32)
            nc.sync.dma_start(pbx[:], pbx_d[:])
            w2t = constp.tile([2, 128], F32)
            nc.sync.dma_start(w2t[:], w2_d[:])
            bdt = constp.tile([128, 128], F32)
            nc.sync.dma_start(bdt[:], bd_d[:])

            for t in range(NR):
                pkrep = pkrepp.tile([128, FD], BF16)
                (nc.sync if t % 2 == 0 else nc.scalar).dma_start(
                    pkrep[:], pkrz_d[t]
                )
                for h in range(2):
                    ring_a = nc.sync if h == 0 else nc.scalar
                    ring_b = nc.scalar if h == 0 else nc.sync

                    xb = xbp.tile([128, MFD], BF16)
                    ring_a.dma_start(
                        xb[:, 0:FD].rearrange("p (dy w) -> p dy w", dy=K),
                        xr[t, 2 * h],
                    )
                    ring_b.dma_start(
                        xb[:, FD:MFD].rearrange("p (dy w) -> p dy w", dy=K),
                        xr[t, 2 * h + 1],
                    )

                    # PROD = xb * pkrep  (pkrep broadcast over the tl pair)
                    prod = prodp.tile([128, MFD], BF16)
                    pr3 = prod.rearrange("p (tl f) -> p tl f", tl=2)
                    xb3 = xb.rearrange("p (tl f) -> p tl f", tl=2)
                    pk3 = pkrep.rearrange("p (a f) -> p a f", a=1)
                    xb3b, pk3b = bass.broadcast_tensor_aps(xb3, pk3)
                    nc.vector.tensor_tensor(pr3, xb3b, pk3b, op=mybir.AluOpType.mult)

                    with nc.allow_low_precision("pairwise bf16 tree adds"):
                        # L1 on Pool: dy 8 -> 4
                        t1 = t1p.tile([128, MFD // 2], BF16)
                        pr4 = prod.rearrange(
                            "p (tl dy q) -> p tl dy q", tl=2, dy=K
                        )
                        t14 = t1.rearrange(
                            "p (tl dy q) -> p tl dy q", tl=2, dy=K // 2
                        )
                        nc.vector.tensor_tensor(
                            t14,
                            pr4[:, :, 0 : K // 2, :],
                            pr4[:, :, K // 2 : K, :],
                            op=mybir.AluOpType.add,
                        )
                        # L2a on DVE: dy 4 -> 2 (natural layout)
                        t2 = t2p.tile([128, MFD // 4], BF16)
                        t14b = t1.rearrange(
                            "p (tl dy q) -> p tl dy q", tl=2, dy=4
                        )
                        t24 = t2.rearrange(
                            "p (tl dy2 q) -> p tl dy2 q", tl=2, dy2=2
                        )
                        nc.vector.tensor_tensor(
                            t24,
                            t14b[:, :, 0:2, :],
                            t14b[:, :, 2:4, :],
                            op=mybir.AluOpType.add,
                        )
                        # L2b on DVE: dy 2 -> 1
                        t3 = t3p.tile([128, MFD // 8], BF16)
                        t34 = t3.rearrange("p (tl a q) -> p tl a q", tl=2, a=1)
                        nc.vector.tensor_tensor(
                            t34,
                            t24[:, :, 0:1, :],
                            t24[:, :, 1:2, :],
                            op=mybir.AluOpType.add,
                        )

                    # f[p, (tl pw)] = sum_dx t3  (single X-axis reduce)
                    f = smallp.tile([128, 128], F32)
                    nc.vector.tensor_reduce(
                        f[:],
                        t3.rearrange("p (q dx) -> p q dx", dx=K),
                        axis=mybir.AxisListType.X,
                        op=mybir.AluOpType.add,
                    )

                    g = gpsum.tile([128, 128], F32)
                    nc.tensor.matmul(g[:], bdt[:], f[:], start=True, stop=False)
                    nc.tensor.matmul(
                        g[:],
                        w2t[:],
                        pbx[:, t * 128 : (t + 1) * 128],
                        start=False,
                        stop=True,
                    )

                    # cast g to bf16 expanded over dx (dense 512-elem inner run)
                    gexp = smallp.tile([128, 128 * K], BF16, tag="gexp")
                    ge3 = gexp.rearrange("p (q dx) -> p q dx", dx=K)
                    gs3 = g.rearrange("p (q a) -> p q a", a=1)
                    ge3b, gs3b = bass.broadcast_tensor_aps(ge3, gs3)
                    nc.scalar.copy(ge3b, gs3b)

                    # OUT = xb * bcast(gexp) over dy, bf16, into PROD's buffer
                    o4 = prod.rearrange("p (tl dy q) -> p tl dy q", tl=2, dy=K)
                    x4 = xb.rearrange("p (tl dy q) -> p tl dy q", tl=2, dy=K)
                    g4 = gexp.rearrange("p (tl a q) -> p tl a q", tl=2, a=1)
                    x4b, g4b = bass.broadcast_tensor_aps(x4, g4)
                    nc.vector.tensor_tensor(o4, x4b, g4b, op=mybir.AluOpType.mult)

                    ring_a.dma_start(
                        outr[t, 2 * h],
                        prod[:, 0:FD].rearrange("p (dy w) -> p dy w", dy=K),
                    )
                    ring_b.dma_start(
                        outr[t, 2 * h + 1],
                        prod[:, FD:MFD].rearrange("p (dy w) -> p dy w", dy=K),
                    )

    nc.compile()
    return nc


_PROGRAM = None
LAST_RESULT = None


def make_in_maps(x, pkr, pb, w2, bd):
    import ml_dtypes

    in_maps = []
    for i in range(8):
        r0 = i * NR
        x_core = (
            np.ascontiguousarray(x[:, :, r0 * K : (r0 + NR) * K, :])
            .astype(ml_dtypes.bfloat16)
            .reshape(NV, 128, NR * K, W)
        )
        pkrz = np.ascontiguousarray(
            np.broadcast_to(pkr[r0 : r0 + NR, None, :], (NR, 128, FD))
        )
        pbx = np.empty((2, NR * 128), np.float32)
        pbx[0] = np.tile(
            pb[r0 * NW : (r0 + NR) * NW].reshape(NR, 1, NW), (1, 2, 1)
        ).reshape(NR * 128)
        pbx[1] = 1.0
        in_maps.append(
            {"x": x_core, "pkrz": pkrz, "pbx": pbx, "w2": w2, "bd": bd}
        )
    return in_maps


def kernel(**inputs):
    global _PROGRAM, LAST_RESULT
    x = np.ascontiguousarray(np.asarray(inputs["x"], dtype=np.float32))
    pkr, pb, w2, bd, pk = _host_tables(
        *[
            np.asarray(inputs[k], dtype=np.float32)
            for k in (
                "w1k", "b1k", "w2k", "b2k",
                "w1b", "b1b", "w2b", "b2b",
                "conv_w", "conv_b",
            )
        ]
    )
    if _PROGRAM is None:
        _PROGRAM = build_program()
    nc = _PROGRAM

    in_maps = make_in_maps(x, pkr, pb, w2, bd)

    conv_w = np.asarray(inputs["conv_w"], np.float64)
    conv_b = np.asarray(inputs["conv_b"], np.float64)

    def _spot_check(out):
        """Verify a sample of patches against the exact host formula;
        catches the rare silent device corruption (bf16 path ~0.4%/elem)."""
        rng = np.random.default_rng(1234)
        worst = 0.0
        for _ in range(32):
            b = int(rng.integers(B))
            ph = int(rng.integers(NH))
            pw = int(rng.integers(NW))
            l = ph * NW + pw
            patch = x[b, :, ph * K : (ph + 1) * K, pw * K : (pw + 1) * K]
            patch = patch.reshape(C, K * K).astype(np.float64)
            feats = patch @ pk[l] + pb[l]
            g = conv_w @ feats + conv_b
            exp = patch * g[:, None]
            got = out[b, :, ph * K : (ph + 1) * K, pw * K : (pw + 1) * K]
            got = got.reshape(C, K * K).astype(np.float64)
            denom = np.linalg.norm(exp) + 1e-30
            worst = max(worst, float(np.linalg.norm(got - exp) / denom))
        return worst

    res = None
    for attempt in range(4):
        try:
            res = run_bass_kernel_spmd(nc, in_maps, list(range(8)))
        except Exception:
            if attempt == 3:
                raise
            continue
        out = np.empty((B, C, H, W), np.float32)
        for i in range(8):
            r0 = i * NR
            out[:, :, r0 * K : (r0 + NR) * K, :] = (
                res.results[i]["out"].astype(np.float32).reshape(B, C, NR * K, W)
            )
        err = _spot_check(out)
        if err < 0.05:
            break
        if attempt == 3:
            raise RuntimeError(f"device output failed spot check ({err:.3f})")
    LAST_RESULT = res
    return out


# revision 21
# speedup vs baseline: 1.4213x; 1.0051x over previous
"""AFPM (adaptive per-patch modulation) kernel for 8 TRN2 NeuronCores.

Reference computation (B=8, C=64, H=W=512, K=8, HID=64):
  - d[l]: normalized distance of each 8x8 patch center from image center
  - pk[l, kk] / pb[l]: tiny MLPs of d (host-precomputable, data-independent)
  - feats[b,c,l] = sum_kk patches[b,c,kk,l] * pk[l,kk] + pb[l]
  - feats2 = conv_w @ feats + conv_b           (1x1 conv over channels)
  - out patches = patches * feats2[:, :, None, :]

Sharding (v4): core i handles patch-rows i*8..i*8+7 for ALL 8 images.
Macro-tile (t, h) = patch-row t (of 8), image-pair-pair h (of 2):
partitions = (u, c); free = (tl, dy, pw, dx) = 8192 covering image
pairs v = 2h, 2h+1.  Halved instruction count vs one-pair tiles.

Engine split per macro-tile (x traffic in bf16; 4 MiB DMA per macro):
  DMA  in   : xb halves, rings alternate sync/scalar
  DVE  mul  : PROD = xb * PKREP[t] (bcast over tl)   TT 2x, ~4.6us
  Pool L1   : dy 8->4 halving add, bf16             (offloads DVE)
  DVE  L2   : dy 4->2, writes (tl,pw,dy2,dx) layout  TT 2x
  DVE  red  : f[p,(tl pw)] = reduce_{dy2,dx}  f32    1x
  PE   g    : g = bd.T @ f (+) w2.T @ [pb;1]         (PSUM accum)
  ACT  gexp : bf16(g) expanded over dx
  DVE  out  : OUT = xb * bcast(gexp) into PROD buf   TT 2x
  DMA  out  : halves on the two rings

pkr rows ship DMA-replicated ([NR,128,FD] bf16, 1 MiB per row, each
reused by both macro-tiles of the row = 4 pair-tiles).
"""

import math
import sys

import numpy as np

for _p in ("/opt/trn_rl_repo",):
    if _p not in sys.path:
        sys.path.insert(0, _p)

import concourse.bass as bass
import concourse.tile as tile
from concourse import bacc, mybir
from concourse.bass_utils import run_bass_kernel_spmd

B, C, H, W, K, HID = 8, 64, 512, 512, 8, 64
NH, NW = H // K, W // K          # 64, 64
L = NH * NW                      # 4096
NR = 8                           # patch-rows per core
NV = 4                           # image-pairs per row
NT = NR * NV                     # 32 pair-tiles per core
FD = K * W                       # 4096 free dim per pair-tile
MFD = 2 * FD                     # 8192 per macro-tile
F32 = mybir.dt.float32
BF16 = mybir.dt.bfloat16

_ERF = np.frompyfunc(math.erf, 1, 1)


def _gelu(x):
    x = np.asarray(x, np.float64)
    return 0.5 * x * (1.0 + _ERF(x / math.sqrt(2.0)).astype(np.float64))


def _host_tables(w1k, b1k, w2k, b2k, w1b, b1b, w2b, b2b, conv_w, conv_b):
    """pk/pb via the tiny MLPs; packed as PKR [NH, FD] plus fold consts."""
    cy = cx = H / 2.0
    max_d = math.sqrt(cy * cy + cx * cx)
    py = np.arange(NH, dtype=np.float64) * K + K / 2.0
    px = np.arange(NW, dtype=np.float64) * K + K / 2.0
    d = np.sqrt((py - cy)[:, None] ** 2 + (px - cx)[None, :] ** 2) / max_d
    d = d.reshape(L, 1)

    pk = _gelu(d @ w1k.astype(np.float64) + b1k) @ w2k.astype(np.float64) + b2k
    pb = (_gelu(d @ w1b.astype(np.float64) + b1b) @ w2b.astype(np.float64) + b2b)[:, 0]

    import ml_dtypes

    # PKR[ph, dy*W + pw*K + dx] = pk[ph*NW + pw, dy*K + dx]  (bf16 on device)
    pkr = (
        pk.reshape(NH, NW, K, K).transpose(0, 2, 1, 3).reshape(NH, FD)
    ).astype(ml_dtypes.bfloat16)

    # g = bd.T @ f  +  w2.T @ [pb_row; 1]   (rank-2 fold of pb and conv_b)
    cw1 = conv_w.astype(np.float64).sum(axis=1)
    w2 = np.stack([np.tile(cw1, 2), np.tile(conv_b.astype(np.float64), 2)]).astype(
        np.float32
    )  # [2, 128]

    bd = np.zeros((128, 128), np.float32)
    bd[0:C, 0:C] = conv_w.T
    bd[C:128, C:128] = conv_w.T
    return pkr, pb, w2, bd, pk


def build_program():
    nc = bacc.Bacc("TRN2", target_bir_lowering=False, debug=False, num_devices=8)
    x_d = nc.dram_tensor("x", [NV, 128, NR * K, W], BF16, kind="ExternalInput")
    pkrz_d = nc.dram_tensor("pkrz", [NR, 128, FD], BF16, kind="ExternalInput")
    pbx_d = nc.dram_tensor("pbx", [2, NR * 128], F32, kind="ExternalInput")
    w2_d = nc.dram_tensor("w2", [2, 128], F32, kind="ExternalInput")
    bd_d = nc.dram_tensor("bd", [128, 128], F32, kind="ExternalInput")
    out_d = nc.dram_tensor("out", [NV, 128, NR * K, W], BF16, kind="ExternalOutput")

    # [t, v, p=(u c), dy, w] views of the DRAM image slices
    xr = x_d.ap().rearrange("v p (t dy) w -> t v p dy w", dy=K)
    outr = out_d.ap().rearrange("v p (t dy) w -> t v p dy w", dy=K)

    with tile.TileContext(nc) as tc:
        with (
            tc.tile_pool(name="const", bufs=1) as constp,
            tc.tile_pool(name="xbp", bufs=4) as xbp,
            tc.tile_pool(name="prodp", bufs=4) as prodp,
            tc.tile_pool(name="t1p", bufs=2) as t1p,
            tc.tile_pool(name="t2p", bufs=2) as t2p,
            tc.tile_pool(name="t3p", bufs=2) as t3p,
            tc.tile_pool(name="pkrepp", bufs=2) as pkrepp,
            tc.tile_pool(name="smallp", bufs=4) as smallp,
            tc.tile_pool(name="gpsum", bufs=3, space="PSUM") as gpsum,
        ):
            pbx = constp.tile([2, NR * 128], F32)
            nc.sync.dma_start(pbx[:], pbx_d[:])
            w2t = constp.tile([2, 128], F32)
            nc.sync.dma_start(w2t[:], w2_d[:])
            bdt = constp.tile([128, 128], F32)
            nc.sync.dma_start(bdt[:], bd_d[:])

            def emit_outmul(st):
                """Deferred modulation+store for a finished macro-tile:
                runs on DVE after the NEXT macro's mul/L1, hiding the
                TR->PE->ACT gexp latency of this macro."""
                prod, xb, gexp, ra, rb, t, h = st
                o4 = prod.rearrange("p (tl dy q) -> p tl dy q", tl=2, dy=K)
                x4 = xb.rearrange("p (tl dy q) -> p tl dy q", tl=2, dy=K)
                g4 = gexp.rearrange("p (tl a q) -> p tl a q", tl=2, a=1)
                x4b, g4b = bass.broadcast_tensor_aps(x4, g4)
                nc.vector.tensor_tensor(o4, x4b, g4b, op=mybir.AluOpType.mult)
                ra.dma_start(
                    outr[t, 2 * h],
                    prod[:, 0:FD].rearrange("p (dy w) -> p dy w", dy=K),
                )
                rb.dma_start(
                    outr[t, 2 * h + 1],
                    prod[:, FD:MFD].rearrange("p (dy w) -> p dy w", dy=K),
                )

            pending = None
            for t in range(NR):
                pkrep = pkrepp.tile([128, FD], BF16)
                (nc.sync if t % 2 == 0 else nc.scalar).dma_start(
                    pkrep[:], pkrz_d[t]
                )
                for h in range(2):
                    ring_a = nc.sync if h == 0 else nc.scalar
                    ring_b = nc.scalar if h == 0 else nc.sync

                    xb = xbp.tile([128, MFD], BF16)
                    ring_a.dma_start(
                        xb[:, 0:FD].rearrange("p (dy w) -> p dy w", dy=K),
                        xr[t, 2 * h],
                    )
                    ring_b.dma_start(
                        xb[:, FD:MFD].rearrange("p (dy w) -> p dy w", dy=K),
                        xr[t, 2 * h + 1],
                    )

                    # PROD = xb * pkrep  (pkrep broadcast over the tl pair)
                    prod = prodp.tile([128, MFD], BF16)
                    pr3 = prod.rearrange("p (tl f) -> p tl f", tl=2)
                    xb3 = xb.rearrange("p (tl f) -> p tl f", tl=2)
                    pk3 = pkrep.rearrange("p (a f) -> p a f", a=1)
                    xb3b, pk3b = bass.broadcast_tensor_aps(xb3, pk3)
                    nc.vector.tensor_tensor(pr3, xb3b, pk3b, op=mybir.AluOpType.mult)

                    with nc.allow_low_precision("pairwise bf16 tree adds"):
                        # L1 on Pool: dy 8 -> 4
                        t1 = t1p.tile([128, MFD // 2], BF16)
                        pr4 = prod.rearrange(
                            "p (tl dy q) -> p tl dy q", tl=2, dy=K
                        )
                        t14 = t1.rearrange(
                            "p (tl dy q) -> p tl dy q", tl=2, dy=K // 2
                        )
                        nc.vector.tensor_tensor(
                            t14,
                            pr4[:, :, 0 : K // 2, :],
                            pr4[:, :, K // 2 : K, :],
                            op=mybir.AluOpType.add,
                        )
                        # L2a on DVE: dy 4 -> 2 (natural layout)
                        t2 = t2p.tile([128, MFD // 4], BF16)
                        t14b = t1.rearrange(
                            "p (tl dy q) -> p tl dy q", tl=2, dy=4
                        )
                        t24 = t2.rearrange(
                            "p (tl dy2 q) -> p tl dy2 q", tl=2, dy2=2
                        )
                        nc.vector.tensor_tensor(
                            t24,
                            t14b[:, :, 0:2, :],
                            t14b[:, :, 2:4, :],
                            op=mybir.AluOpType.add,
                        )
                        # L2b on DVE: dy 2 -> 1
                        t3 = t3p.tile([128, MFD // 8], BF16)
                        t34 = t3.rearrange("p (tl a q) -> p tl a q", tl=2, a=1)
                        nc.vector.tensor_tensor(
                            t34,
                            t24[:, :, 0:1, :],
                            t24[:, :, 1:2, :],
                            op=mybir.AluOpType.add,
                        )

                    # f[p, (tl pw)] = sum_dx t3  (single X-axis reduce)
                    f = smallp.tile([128, 128], F32)
                    nc.vector.tensor_reduce(
                        f[:],
                        t3.rearrange("p (q dx) -> p q dx", dx=K),
                        axis=mybir.AxisListType.X,
                        op=mybir.AluOpType.add,
                    )

                    g = gpsum.tile([128, 128], F32)
                    nc.tensor.matmul(g[:], bdt[:], f[:], start=True, stop=False)
                    nc.tensor.matmul(
                        g[:],
                        w2t[:],
                        pbx[:, t * 128 : (t + 1) * 128],
                        start=False,
                        stop=True,
                    )

                    # modulation of the PREVIOUS macro now: its gexp chain
                    # (TR -> PE -> ACT) finished while DVE ran this macro's
                    # mul + tree, so the outmul issues stall-free.
                    if pending is not None:
                        emit_outmul(pending)

                    # cast g to bf16 expanded over dx (dense 512-elem inner run)
                    gexp = smallp.tile([128, 128 * K], BF16, tag="gexp")
                    ge3 = gexp.rearrange("p (q dx) -> p q dx", dx=K)
                    gs3 = g.rearrange("p (q a) -> p q a", a=1)
                    ge3b, gs3b = bass.broadcast_tensor_aps(ge3, gs3)
                    nc.scalar.copy(ge3b, gs3b)

                    pending = (prod, xb, gexp, ring_a, ring_b, t, h)

            emit_outmul(pending)

    nc.compile()
    return nc


_PROGRAM = None
LAST_RESULT = None


def make_in_maps(x, pkr, pb, w2, bd):
    import ml_dtypes

    in_maps = []
    for i in range(8):
        r0 = i * NR
        x_core = (
            np.ascontiguousarray(x[:, :, r0 * K : (r0 + NR) * K, :])
            .astype(ml_dtypes.bfloat16)
            .reshape(NV, 128, NR * K, W)
        )
        pkrz = np.ascontiguousarray(
            np.broadcast_to(pkr[r0 : r0 + NR, None, :], (NR, 128, FD))
        )
        pbx = np.empty((2, NR * 128), np.float32)
        pbx[0] = np.tile(
            pb[r0 * NW : (r0 + NR) * NW].reshape(NR, 1, NW), (1, 2, 1)
        ).reshape(NR * 128)
        pbx[1] = 1.0
        in_maps.append(
            {"x": x_core, "pkrz": pkrz, "pbx": pbx, "w2": w2, "bd": bd}
        )
    return in_maps


def kernel(**inputs):
    global _PROGRAM, LAST_RESULT
    x = np.ascontiguousarray(np.asarray(inputs["x"], dtype=np.float32))
    pkr, pb, w2, bd, pk = _host_tables(
        *[
            np.asarray(inputs[k], dtype=np.float32)
            for k in (
                "w1k", "b1k", "w2k", "b2k",
                "w1b", "b1b", "w2b", "b2b",
                "conv_w", "conv_b",
            )
        ]
    )
    if _PROGRAM is None:
        _PROGRAM = build_program()
    nc = _PROGRAM

    in_maps = make_in_maps(x, pkr, pb, w2, bd)

    conv_w = np.asarray(inputs["conv_w"], np.float64)
    conv_b = np.asarray(inputs["conv_b"], np.float64)

    def _spot_check(out):
        """Verify a sample of patches against the exact host formula;
        catches the rare silent device corruption (bf16 path ~0.4%/elem)."""
        rng = np.random.default_rng(1234)
        worst = 0.0
        for _ in range(32):
            b = int(rng.integers(B))
            ph = int(rng.integers(NH))
            pw = int(rng.integers(NW))
            l = ph * NW + pw
            patch = x[b, :, ph * K : (ph + 1) * K, pw * K : (pw + 1) * K]
            patch = patch.reshape(C, K * K).astype(np.float64)
            feats = patch @ pk[l] + pb[l]
            g = conv_w @ feats + conv_b
            exp = patch * g[:, None]
            got = out[b, :, ph * K : (ph + 1) * K, pw * K : (pw + 1) * K]
            got = got.reshape(C, K * K).astype(np.float64)
            denom = np.linalg.norm(exp) + 1e-30
            worst = max(worst, float(np.linalg.norm(got - exp) / denom))
        return worst

    res = None
    for attempt in range(4):
        try:
            res = run_bass_kernel_spmd(nc, in_maps, list(range(8)))
        except Exception:
            if attempt == 3:
                raise
            continue
        out = np.empty((B, C, H, W), np.float32)
        for i in range(8):
            r0 = i * NR
            out[:, :, r0 * K : (r0 + NR) * K, :] = (
                res.results[i]["out"].astype(np.float32).reshape(B, C, NR * K, W)
            )
        err = _spot_check(out)
        if err < 0.05:
            break
        if attempt == 3:
            raise RuntimeError(f"device output failed spot check ({err:.3f})")
    LAST_RESULT = res
    return out


# revision 24
# speedup vs baseline: 1.4960x; 1.0526x over previous
"""AFPM (adaptive per-patch modulation) kernel for 8 TRN2 NeuronCores.

Reference computation (B=8, C=64, H=W=512, K=8, HID=64):
  - d[l]: normalized distance of each 8x8 patch center from image center
  - pk[l, kk] / pb[l]: tiny MLPs of d (host-precomputable, data-independent)
  - feats[b,c,l] = sum_kk patches[b,c,kk,l] * pk[l,kk] + pb[l]
  - feats2 = conv_w @ feats + conv_b           (1x1 conv over channels)
  - out patches = patches * feats2[:, :, None, :]

Sharding (v4): core i handles patch-rows i*8..i*8+7 for ALL 8 images.
Macro-tile (t, h) = patch-row t (of 8), image-pair-pair h (of 2):
partitions = (u, c); free = (tl, dy, pw, dx) = 8192 covering image
pairs v = 2h, 2h+1.  Halved instruction count vs one-pair tiles.

Engine split per macro-tile (x traffic in bf16; 4 MiB DMA per macro):
  DMA  in   : xb halves, rings alternate sync/scalar
  DVE  mul  : PROD = xb * PKREP[t] (bcast over tl)   TT 2x, ~4.6us
  Pool L1   : dy 8->4 halving add, bf16             (offloads DVE)
  DVE  L2   : dy 4->2, writes (tl,pw,dy2,dx) layout  TT 2x
  DVE  red  : f[p,(tl pw)] = reduce_{dy2,dx}  f32    1x
  PE   g    : g = bd.T @ f (+) w2.T @ [pb;1]         (PSUM accum)
  ACT  gexp : bf16(g) expanded over dx
  DVE  out  : OUT = xb * bcast(gexp) into PROD buf   TT 2x
  DMA  out  : halves on the two rings

pkr rows ship DMA-replicated ([NR,128,FD] bf16, 1 MiB per row, each
reused by both macro-tiles of the row = 4 pair-tiles).
"""

import math
import sys

import numpy as np

for _p in ("/opt/trn_rl_repo",):
    if _p not in sys.path:
        sys.path.insert(0, _p)

import concourse.bass as bass
import concourse.tile as tile
from concourse import bacc, mybir
from concourse.bass_utils import run_bass_kernel_spmd

B, C, H, W, K, HID = 8, 64, 512, 512, 8, 64
NH, NW = H // K, W // K          # 64, 64
L = NH * NW                      # 4096
NR = 8                           # patch-rows per core
NV = 4                           # image-pairs per row
NT = NR * NV                     # 32 pair-tiles per core
FD = K * W                       # 4096 free dim per pair-tile
MFD = 2 * FD                     # 8192 per macro-tile
F32 = mybir.dt.float32
BF16 = mybir.dt.bfloat16

_ERF = np.frompyfunc(math.erf, 1, 1)


def _gelu(x):
    x = np.asarray(x, np.float64)
    return 0.5 * x * (1.0 + _ERF(x / math.sqrt(2.0)).astype(np.float64))


def _host_tables(w1k, b1k, w2k, b2k, w1b, b1b, w2b, b2b, conv_w, conv_b):
    """pk/pb via the tiny MLPs; packed as PKR [NH, FD] plus fold consts."""
    cy = cx = H / 2.0
    max_d = math.sqrt(cy * cy + cx * cx)
    py = np.arange(NH, dtype=np.float64) * K + K / 2.0
    px = np.arange(NW, dtype=np.float64) * K + K / 2.0
    d = np.sqrt((py - cy)[:, None] ** 2 + (px - cx)[None, :] ** 2) / max_d
    d = d.reshape(L, 1)

    pk = _gelu(d @ w1k.astype(np.float64) + b1k) @ w2k.astype(np.float64) + b2k
    pb = (_gelu(d @ w1b.astype(np.float64) + b1b) @ w2b.astype(np.float64) + b2b)[:, 0]

    import ml_dtypes

    # PKR[ph, dy*W + pw*K + dx] = pk[ph*NW + pw, dy*K + dx]  (bf16 on device)
    pkr = (
        pk.reshape(NH, NW, K, K).transpose(0, 2, 1, 3).reshape(NH, FD)
    ).astype(ml_dtypes.bfloat16)

    # g = bd.T @ f  +  w2.T @ [pb_row; 1]   (rank-2 fold of pb and conv_b)
    cw1 = conv_w.astype(np.float64).sum(axis=1)
    w2 = np.stack([np.tile(cw1, 2), np.tile(conv_b.astype(np.float64), 2)]).astype(
        np.float32
    )  # [2, 128]

    bd = np.zeros((128, 128), np.float32)
    bd[0:C, 0:C] = conv_w.T
    bd[C:128, C:128] = conv_w.T
    return pkr, pb, w2, bd, pk


def build_program():
    nc = bacc.Bacc("TRN2", target_bir_lowering=False, debug=False, num_devices=8)
    x_d = nc.dram_tensor("x", [NV, 128, NR * K, W], BF16, kind="ExternalInput")
    pkrz_d = nc.dram_tensor("pkrz", [NR, 128, FD], BF16, kind="ExternalInput")
    pbx_d = nc.dram_tensor("pbx", [2, NR * 128], F32, kind="ExternalInput")
    w2_d = nc.dram_tensor("w2", [2, 128], F32, kind="ExternalInput")
    bd_d = nc.dram_tensor("bd", [128, 128], F32, kind="ExternalInput")
    out_d = nc.dram_tensor("out", [NV, 128, NR * K, W], BF16, kind="ExternalOutput")

    # [t, v, p=(u c), dy, w] views of the DRAM image slices
    xr = x_d.ap().rearrange("v p (t dy) w -> t v p dy w", dy=K)
    outr = out_d.ap().rearrange("v p (t dy) w -> t v p dy w", dy=K)

    with tile.TileContext(nc) as tc:
        with (
            tc.tile_pool(name="const", bufs=1) as constp,
            tc.tile_pool(name="xbp", bufs=5) as xbp,
            tc.tile_pool(name="prodp", bufs=4) as prodp,
            tc.tile_pool(name="t1p", bufs=2) as t1p,
            tc.tile_pool(name="t2p", bufs=2) as t2p,
            tc.tile_pool(name="t3p", bufs=2) as t3p,
            tc.tile_pool(name="pkrepp", bufs=2) as pkrepp,
            tc.tile_pool(name="smallp", bufs=4) as smallp,
            tc.tile_pool(name="gpsum", bufs=3, space="PSUM") as gpsum,
        ):
            # consts ride the gpsimd SWDGE queue: tiny, needed ~15us in,
            # and this keeps the two HWDGE rings clear for the first tile
            pbx = constp.tile([2, NR * 128], F32)
            nc.gpsimd.dma_start(pbx[:], pbx_d[:])
            w2t = constp.tile([2, 128], F32)
            nc.gpsimd.dma_start(w2t[:], w2_d[:])
            bdt = constp.tile([128, 128], F32)
            nc.gpsimd.dma_start(bdt[:], bd_d[:])

            def emit_outmul(st):
                """Deferred modulation+store for a finished macro-tile:
                runs on DVE after the NEXT macro's mul/L1, hiding the
                TR->PE->ACT gexp latency of this macro."""
                prod, xb, gexp, ra, rb, t, h = st
                o4 = prod.rearrange("p (tl dy q) -> p tl dy q", tl=2, dy=K)
                x4 = xb.rearrange("p (tl dy q) -> p tl dy q", tl=2, dy=K)
                g4 = gexp.rearrange("p (tl a q) -> p tl a q", tl=2, a=1)
                x4b, g4b = bass.broadcast_tensor_aps(x4, g4)
                nc.vector.tensor_tensor(o4, x4b, g4b, op=mybir.AluOpType.mult)
                ra.dma_start(
                    outr[t, 2 * h],
                    prod[:, 0:FD].rearrange("p (dy w) -> p dy w", dy=K),
                )
                rb.dma_start(
                    outr[t, 2 * h + 1],
                    prod[:, FD:MFD].rearrange("p (dy w) -> p dy w", dy=K),
                )

            pending = None
            for t in range(NR):
                pkrep = pkrepp.tile([128, FD], BF16)
                nc.sync.dma_start(pkrep[:, 0 : FD // 2], pkrz_d[t][:, 0 : FD // 2])
                nc.scalar.dma_start(pkrep[:, FD // 2 : FD], pkrz_d[t][:, FD // 2 : FD])
                for h in range(2):
                    ring_a = nc.sync if h == 0 else nc.scalar
                    ring_b = nc.scalar if h == 0 else nc.sync

                    xb = xbp.tile([128, MFD], BF16)
                    ring_a.dma_start(
                        xb[:, 0:FD].rearrange("p (dy w) -> p dy w", dy=K),
                        xr[t, 2 * h],
                    )
                    ring_b.dma_start(
                        xb[:, FD:MFD].rearrange("p (dy w) -> p dy w", dy=K),
                        xr[t, 2 * h + 1],
                    )

                    # PROD = xb * pkrep  (pkrep broadcast over the tl pair)
                    prod = prodp.tile([128, MFD], BF16)
                    pr3 = prod.rearrange("p (tl f) -> p tl f", tl=2)
                    xb3 = xb.rearrange("p (tl f) -> p tl f", tl=2)
                    pk3 = pkrep.rearrange("p (a f) -> p a f", a=1)
                    xb3b, pk3b = bass.broadcast_tensor_aps(xb3, pk3)
                    nc.vector.tensor_tensor(pr3, xb3b, pk3b, op=mybir.AluOpType.mult)

                    with nc.allow_low_precision("pairwise bf16 tree adds"):
                        # L1 on Pool: dy 8 -> 4
                        t1 = t1p.tile([128, MFD // 2], BF16)
                        pr4 = prod.rearrange(
                            "p (tl dy q) -> p tl dy q", tl=2, dy=K
                        )
                        t14 = t1.rearrange(
                            "p (tl dy q) -> p tl dy q", tl=2, dy=K // 2
                        )
                        nc.vector.tensor_tensor(
                            t14,
                            pr4[:, :, 0 : K // 2, :],
                            pr4[:, :, K // 2 : K, :],
                            op=mybir.AluOpType.add,
                        )
                        # L2a on DVE: dy 4 -> 2 (natural layout)
                        t2 = t2p.tile([128, MFD // 4], BF16)
                        t14b = t1.rearrange(
                            "p (tl dy q) -> p tl dy q", tl=2, dy=4
                        )
                        t24 = t2.rearrange(
                            "p (tl dy2 q) -> p tl dy2 q", tl=2, dy2=2
                        )
                        nc.vector.tensor_tensor(
                            t24,
                            t14b[:, :, 0:2, :],
                            t14b[:, :, 2:4, :],
                            op=mybir.AluOpType.add,
                        )
                        # L2b on DVE: dy 2 -> 1
                        t3 = t3p.tile([128, MFD // 8], BF16)
                        t34 = t3.rearrange("p (tl a q) -> p tl a q", tl=2, a=1)
                        nc.vector.tensor_tensor(
                            t34,
                            t24[:, :, 0:1, :],
                            t24[:, :, 1:2, :],
                            op=mybir.AluOpType.add,
                        )

                    # f[p, (tl pw)] = sum_dx t3  (single X-axis reduce)
                    f = smallp.tile([128, 128], F32)
                    nc.vector.tensor_reduce(
                        f[:],
                        t3.rearrange("p (q dx) -> p q dx", dx=K),
                        axis=mybir.AxisListType.X,
                        op=mybir.AluOpType.add,
                    )

                    g = gpsum.tile([128, 128], F32)
                    nc.tensor.matmul(g[:], bdt[:], f[:], start=True, stop=False)
                    nc.tensor.matmul(
                        g[:],
                        w2t[:],
                        pbx[:, t * 128 : (t + 1) * 128],
                        start=False,
                        stop=True,
                    )

                    # modulation of the PREVIOUS macro now: its gexp chain
                    # (TR -> PE -> ACT) finished while DVE ran this macro's
                    # mul + tree, so the outmul issues stall-free.
                    if pending is not None:
                        emit_outmul(pending)

                    # cast g to bf16 expanded over dx (dense 512-elem inner run)
                    gexp = smallp.tile([128, 128 * K], BF16, tag="gexp")
                    ge3 = gexp.rearrange("p (q dx) -> p q dx", dx=K)
                    gs3 = g.rearrange("p (q a) -> p q a", a=1)
                    ge3b, gs3b = bass.broadcast_tensor_aps(ge3, gs3)
                    nc.scalar.copy(ge3b, gs3b)

                    pending = (prod, xb, gexp, ring_a, ring_b, t, h)

            emit_outmul(pending)

    nc.compile()
    return nc


_PROGRAM = None
LAST_RESULT = None


def make_in_maps(x, pkr, pb, w2, bd):
    import ml_dtypes

    in_maps = []
    for i in range(8):
        r0 = i * NR
        x_core = (
            np.ascontiguousarray(x[:, :, r0 * K : (r0 + NR) * K, :])
            .astype(ml_dtypes.bfloat16)
            .reshape(NV, 128, NR * K, W)
        )
        pkrz = np.ascontiguousarray(
            np.broadcast_to(pkr[r0 : r0 + NR, None, :], (NR, 128, FD))
        )
        pbx = np.empty((2, NR * 128), np.float32)
        pbx[0] = np.tile(
            pb[r0 * NW : (r0 + NR) * NW].reshape(NR, 1, NW), (1, 2, 1)
        ).reshape(NR * 128)
        pbx[1] = 1.0
        in_maps.append(
            {"x": x_core, "pkrz": pkrz, "pbx": pbx, "w2": w2, "bd": bd}
        )
    return in_maps


def kernel(**inputs):
    global _PROGRAM, LAST_RESULT
    x = np.ascontiguousarray(np.asarray(inputs["x"], dtype=np.float32))
    pkr, pb, w2, bd, pk = _host_tables(
        *[
            np.asarray(inputs[k], dtype=np.float32)
            for k in (
                "w1k", "b1k", "w2k", "b2k",
                "w1b", "b1b", "w2b", "b2b",
                "conv_w", "conv_b",
            )
        ]
    )
    if _PROGRAM is None:
        _PROGRAM = build_program()
    nc = _PROGRAM

    in_maps = make_in_maps(x, pkr, pb, w2, bd)

    conv_w = np.asarray(inputs["conv_w"], np.float64)
    conv_b = np.asarray(inputs["conv_b"], np.float64)

    def _spot_check(out):
        """Verify a sample of patches against the exact host formula;
        catches the rare silent device corruption (bf16 path ~0.4%/elem)."""
        rng = np.random.default_rng(1234)
        worst = 0.0
        for _ in range(32):
            b = int(rng.integers(B))
            ph = int(rng.integers(NH))
            pw = int(rng.integers(NW))
            l = ph * NW + pw
            patch = x[b, :, ph * K : (ph + 1) * K, pw * K : (pw + 1) * K]
            patch = patch.reshape(C, K * K).astype(np.float64)
            feats = patch @ pk[l] + pb[l]
            g = conv_w @ feats + conv_b
            exp = patch * g[:, None]
            got = out[b, :, ph * K : (ph + 1) * K, pw * K : (pw + 1) * K]
            got = got.reshape(C, K * K).astype(np.float64)
            denom = np.linalg.norm(exp) + 1e-30
            worst = max(worst, float(np.linalg.norm(got - exp) / denom))
        return worst

    res = None
    for attempt in range(4):
        try:
            res = run_bass_kernel_spmd(nc, in_maps, list(range(8)))
        except Exception:
            if attempt == 3:
                raise
            continue
        out = np.empty((B, C, H, W), np.float32)
        for i in range(8):
            r0 = i * NR
            out[:, :, r0 * K : (r0 + NR) * K, :] = (
                res.results[i]["out"].astype(np.float32).reshape(B, C, NR * K, W)
            )
        err = _spot_check(out)
        if err < 0.05:
            break
        if attempt == 3:
            raise RuntimeError(f"device output failed spot check ({err:.3f})")
    LAST_RESULT = res
    return out
